# revision 22
# baseline (speedup 1.0000x reference)
"""Trainium2 Bass kernel for nn_DiffusionTestModel (GCNConv + dense head).

Math (reference):
    A[c, r]  = sym-normalized adjacency (incl. self loops)     [N, N]  (sparse, built dense on host)
    B        = A @ x                                           [N, N]
    aggT     = (B @ gcn_w.T).T = gcn_w @ B.T                   [N, N]
    H1T[k,c] = tanh(aggT[k,c] + gcn_b[k])                      [N, N]
    H2T[e,n] = tanh(sum_k wqT[k,e] H1T[k,n] + wq_b[e])         [E, N]  (E-sharded, never materialized)
    F[e]     = sum_n wf[n] H2T[e,n] + emb[e,:] @ wfe + wf_b    [E]

Device program per core j (SPMD over 8 cores):
    Phase A: BT_s[m, cl]   = sum_r x[r, m] * AT[r, j*SW+cl]    (GCN c-sharded: SW = N/8 cols per core)
             psum evicts convert straight into phase B's SBUF cache (no DRAM
             round trip).
    Phase B: H1T_s[k2, cl] = tanh(sum_m gwt[m, k2] * BT_s[m, cl] + gcn_b[k2])
             AllGather fires per completed 512-row block (NQ=8), overlapping
             B's remaining compute; phase C's first-group caches prefetch as
             the gathered blocks land.
    Phase C: psum[e, n] = sum_k wqt[k, e] * H1T[k, n]  (wq/emb E-sharded; two
             n-shards cached at once to halve wqt re-reads; next group's
             caches prefetch into alternate SBUF slots during compute);
             tanh(+wq_b) fused on evict; f[e] += sum_n wf[n] * H2T-tile
             (DVE fused multiply-reduce, H2 stays in SBUF).

All matmul operands are bf16: fp32r matmuls self-load weights (128 serialized
cycles per 512-cycle matmul); bf16 gets the fast-weight-load path, ~45 ns/MM
faster, and halves every stream/cache/collective byte count. PSUM accumulation
and the final F reduction stay fp32.
"""

import os

import numpy as np
import ml_dtypes

import concourse.bacc as bacc
import concourse.mybir as mybir
import concourse.tile as tile
from concourse.bass_utils import run_bass_kernel_spmd

F32 = mybir.dt.float32
BF16 = mybir.dt.bfloat16
MM_DT = BF16
TANH = mybir.ActivationFunctionType.Tanh
MULT = mybir.AluOpType.mult
ADD = mybir.AluOpType.add

N = 4096          # nodes (= node feature dim)
E = 32768         # edges
EMB = 8
NCORES = 8
ES = E // NCORES  # edges per core
SW = N // NCORES  # GCN column-shard width per core
P = 128


def _ctile(nc, pool, src_ap, slot, k, NB):
    # cache loads go on the vector engine's DMA queue pool so their large
    # transfers never sit head-of-line in front of the matmul stream tiles
    # (which use the sync engine's pool)
    t = pool.tile([P, NB], MM_DT, name="cch", tag=f"c{slot}_{k}")
    nc.gpsimd.dma_start(out=t, in_=src_ap)
    return t


def build_program(n=N, es=ES, ncores=NCORES, MB=512, MBC=256, NQ=8):
    """Build the per-core Bass program (identical across cores; data differs).

    MB: stream width for phases A/B; MBC: for phase C (smaller so C's psum
    block is 4 banks and double-buffers). NQ: AllGather split count.
    """
    nc = bacc.Bacc("TRN2", target_bir_lowering=False, debug=False)
    KT = n // P
    NQ = min(NQ, KT)
    n_etiles = es // P
    sw = n // ncores
    quarter = n // NQ            # rows per AllGather block
    kpq = KT // NQ               # k-tiles per AllGather block
    shared = "Shared" if (ncores > 4 and not os.environ.get("AG_LOCAL")) else "Local"

    x_d = nc.dram_tensor("x", [n, n], MM_DT, kind="ExternalInput")
    at_d = nc.dram_tensor("at", [n, sw], MM_DT, kind="ExternalInput")   # AT[:, my shard]
    gwt_d = nc.dram_tensor("gwt", [n, n], MM_DT, kind="ExternalInput")
    gbt_d = nc.dram_tensor("gbt", [P, KT], F32, kind="ExternalInput")
    wqt_d = nc.dram_tensor("wqt", [n, es], MM_DT, kind="ExternalInput")
    wqbt_d = nc.dram_tensor("wqbt", [P, n_etiles], F32, kind="ExternalInput")
    wfn_d = nc.dram_tensor("wfn", [P, n], F32, kind="ExternalInput")
    embr_d = nc.dram_tensor("embr", [P, n_etiles * EMB], F32, kind="ExternalInput")
    wfe_d = nc.dram_tensor("wfe", [P, EMB], F32, kind="ExternalInput")
    wfb_d = nc.dram_tensor("wfb", [P, 1], F32, kind="ExternalInput")
    out_d = nc.dram_tensor("out", [P, n_etiles], F32, kind="ExternalOutput")

    with tile.TileContext(nc) as tc:
        with tc.tile_pool(name="dram", bufs=1, space="DRAM") as dram, \
             tc.tile_pool(name="cachep", bufs=1) as cp, \
             tc.tile_pool(name="streamp", bufs=12) as sp, \
             tc.tile_pool(name="psump", bufs=2, space="PSUM") as pp, \
             tc.tile_pool(name="evictp", bufs=2) as ep, \
             tc.tile_pool(name="constp", bufs=1) as constp:

            ag_in = dram.tile([n, sw], MM_DT, name="ag_in")
            h1t_q = [dram.tile([ncores * quarter, sw], MM_DT, name=f"h1t_q{h}",
                               addr_space=shared) for h in range(NQ)]

            # ---------- constants (loaded up front) ----------
            gbt_sb = constp.tile([P, KT], F32, name="gbt_sb")
            nc.sync.dma_start(out=gbt_sb[:], in_=gbt_d[:, :])
            wqbt_sb = constp.tile([P, n_etiles], F32, name="wqbt_sb")
            nc.sync.dma_start(out=wqbt_sb[:], in_=wqbt_d[:, :])
            wfn_sb = constp.tile([P, n], F32, name="wfn_sb")
            nc.sync.dma_start(out=wfn_sb[:], in_=wfn_d[:, :])
            f_acc = constp.tile([P, n_etiles], F32, name="f_acc")
            nc.vector.memset(f_acc[:], 0.0)

            # cache slots: 4 x KT tiles of [P, sw] bf16 (4 MB each slot).
            # slot 0: phase A's AT cache, then phase C groups 0/2 shard-even
            # slot 1: phase B's BT cache, then phase C groups 1/3 shard-even
            # slots 2/3: phase C shard-odd for groups 0/2 and 1/3
            cacheA = [_ctile(nc, cp, at_d[k * P:(k + 1) * P, :], 0, k, sw)
                      for k in range(KT)]
            cacheB = [cp.tile([P, sw], MM_DT, name="cch", tag=f"c1_{k}")
                      for k in range(KT)]

            # ---------- Phase A: BT_s = x.T @ AT_s ----------
            # evict converts psum straight into phase B's bf16 cache tiles.
            for mb in range(n // MB):
                psums = [pp.tile([P, sw], F32, name="a_ps", tag=f"ps{i}")
                         for i in range(MB // P)]
                for k in range(KT):
                    st = sp.tile([P, MB], MM_DT, name="a_st", tag="stream")
                    nc.sync.dma_start(out=st, in_=x_d[k * P:(k + 1) * P, mb * MB:(mb + 1) * MB])
                    for i in range(MB // P):
                        nc.tensor.matmul(
                            out=psums[i][:],
                            lhsT=st[:, i * P:(i + 1) * P],
                            rhs=cacheA[k][:],
                            start=(k == 0),
                            stop=(k == KT - 1),
                        )
                for i in range(MB // P):
                    m2 = mb * (MB // P) + i
                    nc.vector.tensor_copy(out=cacheB[m2][:], in_=psums[i][:])

            # ---------- Phase B: H1T_s = tanh(gwt.T @ BT_s + gcn_b) ----------
            # AllGather of each finished 512-row block overlaps the rest of B;
            # phase C's first-group caches (slots 0 and 2) prefetch as the
            # gathered blocks land.
            def c_tile_src(s, k):
                h, r = divmod(k, kpq)
                return h1t_q[h][s * quarter + r * P: s * quarter + (r + 1) * P, :]

            cacheC = {}   # (shard, k) -> tile

            for mb in range(n // MB):
                psums = [pp.tile([P, sw], F32, name="b_ps", tag=f"ps{i}")
                         for i in range(MB // P)]
                for k in range(KT):
                    st = sp.tile([P, MB], MM_DT, name="b_st", tag="stream")
                    nc.sync.dma_start(out=st, in_=gwt_d[k * P:(k + 1) * P, mb * MB:(mb + 1) * MB])
                    for i in range(MB // P):
                        nc.tensor.matmul(
                            out=psums[i][:],
                            lhsT=st[:, i * P:(i + 1) * P],
                            rhs=cacheB[k][:],
                            start=(k == 0),
                            stop=(k == KT - 1),
                        )
                for i in range(MB // P):
                    k2t = mb * (MB // P) + i
                    sb = ep.tile([P, sw], MM_DT, name="b_ev", tag="b_ev")
                    nc.scalar.activation(sb[:], psums[i][:], TANH,
                                         bias=gbt_sb[:, k2t:k2t + 1])
                    nc.gpsimd.dma_start(
                        out=ag_in[mb * MB + i * P: mb * MB + (i + 1) * P, :],
                        in_=sb[:])
                done_rows = (mb + 1) * MB
                for h in range(NQ):
                    if done_rows - MB < (h + 1) * quarter <= done_rows:
                        nc.gpsimd.collective_compute(
                            "AllGather", mybir.AluOpType.bypass,
                            ins=[ag_in[h * quarter:(h + 1) * quarter, :]],
                            outs=[h1t_q[h][:]],
                            replica_groups=[list(range(ncores))],
                        )
                        # prefetch phase C group-0 cache tiles for the block
                        # gathered one step earlier (lets the DMA queue stay
                        # ahead without waiting on the fresh AllGather)
                        if h >= 1:
                            for k in range((h - 1) * kpq, h * kpq):
                                cacheC[(0, k)] = _ctile(nc, cp, c_tile_src(0, k), 0, k, sw)
                                cacheC[(1, k)] = _ctile(nc, cp, c_tile_src(1, k), 2, k, sw)
            for k in range((NQ - 1) * kpq, NQ * kpq):
                cacheC[(0, k)] = _ctile(nc, cp, c_tile_src(0, k), 0, k, sw)
                cacheC[(1, k)] = _ctile(nc, cp, c_tile_src(1, k), 2, k, sw)

            # ---------- Phase C: head (H2 stays on-chip) ----------
            # groups of 2 shards; group g uses slots (0,2) when g even,
            # (1,3) when g odd; next group's tiles prefetch during compute.
            CGROUP = 2
            n_groups = ncores // CGROUP
            slot_pairs = [(0, 2), (1, 3)]

            def evict_c(psum_ap, shard, mb, i):
                et = mb * (MBC // P) + i
                n0 = shard * sw
                # DVE moves psum->SBUF (ACT reading PSUM while the PE writes
                # other banks slows the MM stream; DVE's PSUM path doesn't)
                raw = ep.tile([P, sw], F32, name="c_raw", tag="c_raw")
                nc.vector.tensor_copy(out=raw[:], in_=psum_ap)
                h2 = ep.tile([P, sw], F32, name="c_h2", tag="c_h2")
                nc.scalar.activation(h2[:], raw[:], TANH,
                                     bias=wqbt_sb[:, et:et + 1])
                scr = ep.tile([P, sw], F32, name="c_scr", tag="c_scr")
                fpart = ep.tile([P, 1], F32, name="c_fp", tag="c_fp")
                nc.vector.scalar_tensor_tensor(
                    out=scr[:], in0=h2[:], scalar=1.0,
                    in1=wfn_sb[:, n0:n0 + sw],
                    op0=MULT, op1=MULT, accum_out=fpart[:])
                nc.vector.tensor_add(f_acc[:, et:et + 1],
                                     f_acc[:, et:et + 1], fpart[:])

            n_mbc = es // MBC
            for gi in range(n_groups):
                shards = (2 * gi, 2 * gi + 1)
                sl_even, sl_odd = slot_pairs[gi % 2]
                nsl_even, nsl_odd = slot_pairs[(gi + 1) % 2]
                nxt = (2 * (gi + 1), 2 * (gi + 1) + 1) if gi + 1 < n_groups else None
                for mb in range(n_mbc):
                    psums = [[pp.tile([P, sw], F32, name="c_ps",
                                      tag=f"ps{i * 2 + s}")
                              for s in range(CGROUP)] for i in range(MBC // P)]
                    for kk in range(KT // 2):
                        # two k-tiles per stream DMA: same 1 KB/partition tile
                        # geometry (and pool) as phase A's stream tiles
                        st = sp.tile([P, 2, MBC], MM_DT, name="a_st", tag="stream")
                        nc.sync.dma_start(
                            out=st,
                            in_=wqt_d[kk * 2 * P:(kk * 2 + 2) * P,
                                      mb * MBC:(mb + 1) * MBC].rearrange(
                                          "(j p) c -> p j c", j=2, p=P))
                        for j in range(2):
                            k = kk * 2 + j
                            for i in range(MBC // P):
                                for s in range(CGROUP):
                                    nc.tensor.matmul(
                                        out=psums[i][s][:],
                                        lhsT=st[:, j, i * P:(i + 1) * P],
                                        rhs=cacheC[(shards[s], k)][:],
                                        start=(k == 0),
                                        stop=(k == KT - 1),
                                    )
                    for i in range(MBC // P):
                        for s in range(CGROUP):
                            evict_c(psums[i][s][:], shards[s], mb, i)
                    # spread next group's cache loads across this group's mbs
                    if nxt is not None:
                        for k in (2 * mb, 2 * mb + 1):
                            if k < KT:
                                cacheC[(nxt[0], k)] = _ctile(
                                    nc, cp, c_tile_src(nxt[0], k), nsl_even, k, sw)
                                cacheC[(nxt[1], k)] = _ctile(
                                    nc, cp, c_tile_src(nxt[1], k), nsl_odd, k, sw)

            # edge-embedding contribution + wf_b, then write out
            embr_sb = constp.tile([P, n_etiles * EMB], F32, name="embr_sb")
            nc.sync.dma_start(out=embr_sb[:], in_=embr_d[:, :])
            wfe_sb = constp.tile([P, EMB], F32, name="wfe_sb")
            nc.sync.dma_start(out=wfe_sb[:], in_=wfe_d[:, :])
            wfb_sb = constp.tile([P, 1], F32, name="wfb_sb")
            nc.sync.dma_start(out=wfb_sb[:], in_=wfb_d[:, :])
            scr9 = constp.tile([P, EMB], F32, name="c_scr9")
            fp9 = constp.tile([P, 1], F32, name="c_fp9")
            for t in range(n_etiles):
                nc.vector.scalar_tensor_tensor(
                    out=scr9[:], in0=embr_sb[:, t * EMB:(t + 1) * EMB],
                    scalar=1.0, in1=wfe_sb[:],
                    op0=MULT, op1=MULT, accum_out=fp9[:])
                nc.vector.tensor_add(f_acc[:, t:t + 1],
                                     f_acc[:, t:t + 1], fp9[:])
            out_sb = constp.tile([P, n_etiles], F32, name="out_sb")
            nc.vector.tensor_scalar_add(out_sb[:], f_acc[:], wfb_sb[:, 0:1])
            nc.sync.dma_start(out=out_d[:, :], in_=out_sb[:])

    nc.finalize()
    return nc


def host_inputs(x, edge_index, edge_weight, gcn_w, gcn_b, wq_w, wq_b, emb,
                wf_w, wf_b, n=N, e=E, ncores=NCORES):
    """Build the per-core input maps (host-side preprocessing)."""
    es = e // ncores
    sw = n // ncores
    n_etiles = es // P
    kt = n // P
    bf16 = ml_dtypes.bfloat16
    x = np.ascontiguousarray(np.asarray(x, dtype=np.float32).astype(bf16))
    row = np.asarray(edge_index[0], dtype=np.int64)
    col = np.asarray(edge_index[1], dtype=np.int64)
    ew = np.asarray(edge_weight, dtype=np.float32)

    deg = np.zeros(n, dtype=np.float32)
    np.add.at(deg, col, ew)
    deg += 1.0  # self loops, weight 1
    dis = (1.0 / np.sqrt(deg)).astype(np.float32)

    at = np.zeros((n, n), dtype=np.float32)
    np.add.at(at, (row, col), dis[row] * ew * dis[col])
    idx = np.arange(n)
    at[idx, idx] += dis * dis
    at = at.astype(bf16)

    gwt = np.ascontiguousarray(np.asarray(gcn_w, dtype=np.float32).T.astype(bf16))
    gbt = np.ascontiguousarray(np.asarray(gcn_b, dtype=np.float32).reshape(kt, P).T)
    wfn = np.ascontiguousarray(np.broadcast_to(
        np.asarray(wf_w[0, :n], dtype=np.float32), (P, n)))
    wfe = np.ascontiguousarray(np.broadcast_to(
        np.asarray(wf_w[0, n:n + EMB], dtype=np.float32), (P, EMB)))
    wfb = np.full((P, 1), np.float32(np.asarray(wf_b).reshape(-1)[0]), dtype=np.float32)

    wq_w = np.asarray(wq_w, dtype=np.float32)
    wq_b = np.asarray(wq_b, dtype=np.float32)
    emb = np.asarray(emb, dtype=np.float32)

    in_maps = []
    for j in range(ncores):
        sl = slice(j * es, (j + 1) * es)
        wqt = np.ascontiguousarray(wq_w[sl, :].T.astype(bf16))
        wqbt = np.ascontiguousarray(wq_b[sl].reshape(n_etiles, P).T)
        embr = np.ascontiguousarray(
            emb[sl].reshape(n_etiles, P, EMB).transpose(1, 0, 2).reshape(P, n_etiles * EMB))
        at_s = np.ascontiguousarray(at[:, j * sw:(j + 1) * sw])
        in_maps.append(dict(x=x, at=at_s, gwt=gwt, gbt=gbt, wqt=wqt, wqbt=wqbt,
                            wfn=wfn, embr=embr, wfe=wfe, wfb=wfb))
    return in_maps


_PROG = None


def kernel(**inputs):
    global _PROG
    in_maps = host_inputs(**inputs)
    if _PROG is None:
        _PROG = build_program()
    res = run_bass_kernel_spmd(_PROG, in_maps, core_ids=list(range(NCORES)))
    shards = [res.results[j]["out"].T.ravel() for j in range(NCORES)]
    return np.concatenate(shards).astype(np.float32)
